# revision 8
# baseline (speedup 1.0000x reference)
"""Trainium2 Bass kernel for nn_LinearAttentionBlock (linear attention).

Per-core (data-parallel over batch, 1 batch / core):
  x_b [4096, 512] -> qkv = x_b @ w_qkv -> per-head LayerNorm(q), LayerNorm(k)
  dots_h = LN(k)_h^T @ v_h   [64, 64]
  out_h  = LN(q)_h @ dots_h / 4096
  out    = concat_h(out_h)   [4096, 512]

v3 design (vs v2 at 202us, v1 at 218us):
  - Prologue on HWDGE: w and x chunks 0/1 loaded f32 (fast HW descriptors),
    converted to bf16 on-chip; per-chunk pipelined weight centering. First
    real matmul at ~10us instead of ~32us.
  - Engine rebalance: ACT evacuates q/k/v from PSUM to bf16 SBUF; GpSimd
    (otherwise idle) computes the squares and the rstd multiplies; DVE only
    does the segmented reduces + reciprocal.
  - Software pipeline: stats for tile i, rstd-muls for tile i-1, dots/sumv
    matmuls for tile i-2; x loads 4 chunks ahead, xT transposes 2 chunks
    ahead, qhat^T transposes 2 steps late -- every queue's FIFO head has
    pre-satisfied waits, so the PE never idles (HAM stays at full clock).
  - All xbar transposes serialized on the sync queue (concurrent transposes
    on two HWDGE queues corrupt packets -- found in v2).
  - Warm-up + keepalive dummy matmuls cover prologue and fixup phases.
  - beta_q correction: rank-1 K=1 matmul into the pass-3 PSUM group;
    evacuation alternates DVE/ACT plain copies.
"""
import threading

import numpy as np

import concourse.bacc as bacc
import concourse.bass as bass
import concourse.mybir as mybir
from concourse.tile import TileContext
from concourse.tile_rust import add_dep_helper

P = 128
NTOK = 4096          # tokens per batch (64*64)
CIN = 512            # input channels
N3 = 3 * CIN         # qkv columns
MT = NTOK // P       # 32 m-tiles
KC = CIN // P        # 4 k-chunks
H = 8                # heads
D = 64               # dim per head
NPAIR = H // 2       # 4 head pairs
CH = 4               # m-tiles per chunk
NCH = MT // CH       # 8 chunks
NCORES = 8
LN_EPS = 1e-5

f32 = mybir.dt.float32
bf16 = mybir.dt.bfloat16
X = mybir.AxisListType.X
MUL = mybir.AluOpType.mult
SUB = mybir.AluOpType.subtract
ADD = mybir.AluOpType.add


def _bc(ap, n):
    """Append a stride-0 broadcast dim of size n to an AP."""
    return bass.AP(ap.tensor, ap.offset, list(ap.ap) + [[0, n]])


def _col64(dram_ap):
    """View a [64] DRAM tensor as a [64, 1] column AP (partition-major)."""
    return bass.AP(dram_ap.tensor, dram_ap.offset, [[1, D], [1, 1]])


def _body(nc, tc, pools, x, w, gq, bq, gk, bk, out):
    singles, xch, xTp, qhp, sqp, stp, kvp, outp = pools

    # ---------------- P0: constants + weight prep ----------------
    ones_bf = singles.tile([P, P], bf16)
    nc.vector.memset(ones_bf[:], 1.0)

    # w: 4 chunked HWDGE f32 loads on the scalar queue, prep pipelined
    w_f32 = singles.tile([P, KC, N3], f32)
    for c in range(KC):
        nc.scalar.dma_start(
            out=w_f32[:, c, :],
            in_=w[c * P:(c + 1) * P, :])

    # x chunk staging (prologue chunks 0/1 via HWDGE f32 + ACT convert)
    x_tiles = {}
    xT_tiles = {}
    xst0 = singles.tile([P, CH, CIN], f32)
    xst1 = singles.tile([P, CH, CIN], f32)
    for j, xst in ((0, xst0), (1, xst1)):
        nc.sync.dma_start(
            out=xst[:],
            in_=x[j * CH * P:(j + 1) * CH * P, :].rearrange(
                "(t p) k -> p t k", p=P))
        t = xch.tile([P, CH, CIN], bf16, tag="x", name="x_ch")
        nc.scalar.copy(t[:], xst[:])
        x_tiles[j] = t

    def load_chunk(j):
        t = xch.tile([P, CH, CIN], bf16, tag="x", name="x_ch")
        nc.gpsimd.dma_start(
            out=t[:],
            in_=x[j * CH * P:(j + 1) * CH * P, :].rearrange(
                "(t p) k -> p t k", p=P))
        x_tiles[j] = t

    def transpose_chunk(j):
        t = xTp.tile([P, CH, KC, P], bf16, tag="xT", name="xT")
        nc.sync.dma_start(out=t[:], in_=x_tiles.pop(j)[:], transpose=True)
        xT_tiles[j] = t

    load_chunk(2)
    load_chunk(3)

    # per-chunk weight centering: wbar -> w_qk (bf16), w_v (bf16)
    wbar = singles.tile([P, KC, 2 * H], f32)
    w_qk = singles.tile([P, KC, 2 * CIN], bf16)
    w_v = singles.tile([P, KC, CIN], bf16)
    for c in range(KC):
        nc.vector.reduce_sum(
            wbar[:, c, :],
            w_f32[:, c, 0:2 * CIN].rearrange("p (g d) -> p g d", d=D),
            axis=X)
        nc.vector.tensor_scalar_mul(out=wbar[:, c, :], in0=wbar[:, c, :],
                                    scalar1=1.0 / D)
        nc.vector.tensor_tensor(
            out=w_qk[:, c, :].rearrange("p (g d) -> p g d", d=D),
            in0=w_f32[:, c, 0:2 * CIN].rearrange("p (g d) -> p g d", d=D),
            in1=_bc(wbar[:, c, :], D),
            op=SUB)
        nc.scalar.copy(w_v[:, c, :], w_f32[:, c, 2 * CIN:])

    transpose_chunk(0)
    transpose_chunk(1)

    # gamma/beta columns replicated onto both partition halves
    gq2 = singles.tile([P, 1], f32)
    gk2 = singles.tile([P, 1], f32)
    bk2 = singles.tile([P, 1], f32)
    for half in (0, 1):
        sl = slice(half * D, (half + 1) * D)
        nc.sync.dma_start(out=gq2[sl, :], in_=_col64(gq))
        nc.sync.dma_start(out=gk2[sl, :], in_=_col64(gk))
        nc.sync.dma_start(out=bk2[sl, :], in_=_col64(bk))
    bq_bf = singles.tile([D, 1], bf16)
    nc.gpsimd.dma_start(out=bq_bf[:], in_=_col64(bq))

    eps_t = singles.tile([P, 1], f32)
    nc.vector.memset(eps_t[:], float(D) * LN_EPS)
    d_all = singles.tile([P, NPAIR, P], bf16)
    nc.gpsimd.memset(d_all[:], 0.0)

    qhatT = singles.tile([P, NCH, CH, KC, P], bf16)

    with tc.tile_pool(name="ps_acc", bufs=1, space="PSUM") as ps_acc:
        dots_ps = ps_acc.tile([P, 4 * P], f32)
        sumv_ps = ps_acc.tile([P, CIN], f32)
        with tc.tile_pool(name="ps_qkv", bufs=2, space="PSUM") as ps_qkv:
            # HAM warm-up burst (no data deps -> runs immediately), then
            # keepalives pinned to late-prologue producers. Targets the
            # sumv bank; the real sumv group re-clears it with start=True.
            for _ in range(44):
                nc.tensor.matmul(sumv_ps[:, 0:P], lhsT=ones_bf[:],
                                 rhs=ones_bf[:], start=True, stop=True)
            for c in range(KC):
                for _ in range(2):
                    nc.tensor.matmul(sumv_ps[:, 0:P], lhsT=ones_bf[:],
                                     rhs=w_qk[:, c, 0:P],
                                     start=True, stop=True)
            for _ in range(2):
                nc.tensor.matmul(sumv_ps[:, 0:P], lhsT=ones_bf[:],
                                 rhs=xT_tiles[0][:, 0, 0, :],
                                 start=True, stop=True)

            _p1_loop(nc, load_chunk, transpose_chunk, xT_tiles,
                     w_qk, w_v, eps_t, ones_bf, qhatT,
                     dots_ps, sumv_ps, ps_qkv, qhp, sqp, stp, kvp)

        # ---------------- P2: dots fixups ----------------
        dots_sb = singles.tile([P, 4 * P], f32)
        nc.vector.tensor_copy(out=dots_sb[:], in_=dots_ps[:])
        sumv_sb = singles.tile([P, CIN], f32)
        nc.vector.tensor_copy(out=sumv_sb[:], in_=sumv_ps[:])

    ktmp = singles.tile([P, NPAIR, D], f32)
    bsum = singles.tile([P, NPAIR, D], f32)
    deo = singles.tile([P, NPAIR, D], f32)
    for half in (0, 1):
        sl = slice(half * D, (half + 1) * D)
        # KV diag block, scaled by gamma_k * 8
        nc.vector.tensor_scalar(
            out=ktmp[sl, :, :],
            in0=dots_sb[sl, :].rearrange("p (pr x) -> p pr x", x=P)[
                :, :, half * D:(half + 1) * D],
            scalar1=gk2[sl, :], scalar2=8.0, op0=MUL, op1=MUL)
        # beta_k (x) sumV
        nc.vector.tensor_scalar(
            out=bsum[sl, :, :],
            in0=sumv_sb[sl, :].rearrange(
                "p (pr two d) -> p pr two d", two=2, d=D)[:, :, half, :],
            scalar1=bk2[sl, :], scalar2=None, op0=MUL)
    nc.vector.tensor_add(deo[:], ktmp[:], bsum[:])

    for half in (0, 1):
        sl = slice(half * D, (half + 1) * D)
        nc.vector.tensor_scalar(
            out=d_all[sl, :, half * D:(half + 1) * D],
            in0=deo[sl, :, :],
            scalar1=gq2[sl, :], scalar2=8.0 / NTOK, op0=MUL, op1=MUL)

    # c row: beta_q @ dots / NTOK (rank-1 correction, added in pass 3)
    dstack = singles.tile([D, H, D], bf16)
    nc.vector.tensor_copy(
        out=dstack.rearrange("p (pr two) d -> p pr two d", two=2)[:, :, 0, :],
        in_=deo[0:D, :, :])
    nc.gpsimd.dma_start(
        out=dstack.rearrange("p (pr two) d -> p pr two d", two=2)[:, :, 1, :],
        in_=deo[D:P, :, :])

    with tc.tile_pool(name="ps_fix", bufs=1, space="PSUM") as ps_fix, \
         tc.tile_pool(name="ps_out", bufs=2, space="PSUM") as ps_out, \
         tc.tile_pool(name="ps_warm2", bufs=1, space="PSUM") as ps_warm2:
        # keepalive through the fixup phase (no >3.4us PE idle)
        warm2 = ps_warm2.tile([P, P], f32)
        nc.tensor.matmul(warm2[:], lhsT=ones_bf[:], rhs=d_all[:, 0, :],
                         start=True, stop=True)

        c_ps = ps_fix.tile([1, CIN], f32)
        nc.tensor.matmul(c_ps[:], lhsT=bq_bf[:],
                         rhs=dstack.rearrange("p h d -> p (h d)"),
                         start=True, stop=True)
        c_bf = singles.tile([1, CIN], bf16)
        nc.vector.tensor_scalar_mul(out=c_bf[:], in0=c_ps[:],
                                    scalar1=1.0 / NTOK)
        nc.tensor.matmul(warm2[0:1, :], lhsT=ones_bf[0:1, 0:1],
                         rhs=c_bf[:, 0:P], start=True, stop=True)

        # ------------ P3: out = qhat @ D (pair blockdiag) + 1 (x) c ------
        for ci in range(NCH):
            out_ch = outp.tile([P, CH, CIN], f32, tag="out", name="out_ch")
            for tt in range(CH):
                o_ps = ps_out.tile([P, CIN], f32, tag="o", name="o_ps")
                mm0 = None
                for pr in range(NPAIR):
                    mm = nc.tensor.matmul(
                        o_ps[:, pr * P:(pr + 1) * P],
                        lhsT=qhatT[:, ci, tt, pr, :],
                        rhs=d_all[:, pr, :],
                        start=(pr == 0), stop=False)
                    if pr == 0:
                        mm0 = mm
                    else:
                        add_dep_helper(mm.ins, mm0.ins, sync=False,
                                       reason="psum group start order")
                mmc = nc.tensor.matmul(
                    o_ps[:], lhsT=ones_bf[0:1, :], rhs=c_bf[:],
                    start=False, stop=True)
                add_dep_helper(mmc.ins, mm0.ins, sync=False,
                               reason="psum group start order")
                if tt % 2 == 0:
                    nc.vector.tensor_copy(out=out_ch[:, tt, :], in_=o_ps[:])
                else:
                    nc.scalar.copy(out_ch[:, tt, :], o_ps[:])
            nc.sync.dma_start(
                out=out[ci * CH * P:(ci + 1) * CH * P, :].rearrange(
                    "(t p) k -> p t k", p=P),
                in_=out_ch[:])


def _p1_loop(nc, load_chunk, transpose_chunk, xT_tiles,
             w_qk, w_v, eps_t, ones_bf, qhatT,
             dots_ps, sumv_ps, ps_qkv, qhp, sqp, stp, kvp):
    q_sb_t = {}
    k_sb_t = {}
    v_ps_t = {}
    rstd_t = {}
    khat_t = {}
    v_bf_t = {}
    qh_ch = {}
    dots_mm0 = None

    for i in range(MT + 2):
        # chunk machinery: loads 4 chunks ahead, xT 2 chunks ahead,
        # qhat^T transposes 2 steps after their chunk completes.
        if i % CH == 0 and i < MT:
            jj = i // CH
            if jj + 4 < NCH:
                load_chunk(jj + 4)
            if jj + 2 < NCH:
                transpose_chunk(jj + 2)
        if i >= 6 and (i - 6) % CH == 0:
            jdone = (i - 6) // CH
            if jdone in qh_ch:
                nc.sync.dma_start(out=qhatT[:, jdone],
                                  in_=qh_ch.pop(jdone)[:], transpose=True)

        if i < MT:
            j, tt = divmod(i, CH)
            if tt == 0:
                qh_ch[j] = qhp.tile([P, CH, CIN], bf16, tag="qh",
                                    name="qh_ch")

            # ---- QKV matmuls for tile i ----
            xT = xT_tiles[j]
            psq = ps_qkv.tile([P, CIN], f32, tag="q", name="q_ps")
            psk = ps_qkv.tile([P, CIN], f32, tag="k", name="k_ps")
            psv = v_ps_t[i] = ps_qkv.tile([P, CIN], f32, tag="v", name="v_ps")
            for pst, rhs_tile, base in ((psq, w_qk, 0),
                                        (psk, w_qk, CIN),
                                        (psv, w_v, 0)):
                for c in range(KC):
                    nc.tensor.matmul(
                        pst[:], lhsT=xT[:, tt, c, :],
                        rhs=rhs_tile[:, c, base:base + CIN],
                        start=(c == 0), stop=(c == KC - 1))

            # ---- evacuate q/k (ACT), squares (GpSimd), stats (DVE) ----
            q_sb = q_sb_t[i] = sqp.tile([P, CIN], bf16, tag="q_sb",
                                        name="q_sb")
            k_sb = k_sb_t[i] = sqp.tile([P, CIN], bf16, tag="k_sb",
                                        name="k_sb")
            nc.scalar.copy(q_sb[:], psq[:])
            nc.scalar.copy(k_sb[:], psk[:])
            sq_q = sqp.tile([P, CIN], bf16, tag="sq_q", name="sq_q")
            sq_k = sqp.tile([P, CIN], bf16, tag="sq_k", name="sq_k")
            nc.gpsimd.tensor_tensor(out=sq_q[:], in0=q_sb[:], in1=q_sb[:],
                                    op=MUL)
            nc.gpsimd.tensor_tensor(out=sq_k[:], in0=k_sb[:], in1=k_sb[:],
                                    op=MUL)
            st = stp.tile([P, 2, H], f32, tag="st", name="st")
            nc.vector.reduce_sum(
                st[:, 0, :], sq_q.rearrange("p (h d) -> p h d", d=D), axis=X)
            nc.vector.reduce_sum(
                st[:, 1, :], sq_k.rearrange("p (h d) -> p h d", d=D), axis=X)
            rstd = rstd_t[i] = stp.tile([P, 2, H], f32, tag="rstd",
                                        name="rstd")
            nc.scalar.activation(
                out=rstd[:], in_=st[:],
                func=mybir.ActivationFunctionType.Sqrt,
                bias=eps_t[:], scale=1.0)
            nc.vector.reciprocal(rstd[:], rstd[:])

        # ---- rstd application for tile i-1 (GpSimd) + v evac (ACT) ----
        m = i - 1
        if 0 <= m < MT:
            jm, ttm = divmod(m, CH)
            rstd = rstd_t.pop(m)
            nc.gpsimd.tensor_tensor(
                out=qh_ch[jm][:, ttm, :].rearrange("p (h d) -> p h d", d=D),
                in0=q_sb_t.pop(m).rearrange("p (h d) -> p h d", d=D),
                in1=_bc(rstd[:, 0, :], D), op=MUL)
            khat = khat_t[m] = kvp.tile([P, CIN], bf16, tag="khat",
                                        name="khat")
            nc.gpsimd.tensor_tensor(
                out=khat.rearrange("p (h d) -> p h d", d=D),
                in0=k_sb_t.pop(m).rearrange("p (h d) -> p h d", d=D),
                in1=_bc(rstd[:, 1, :], D), op=MUL)
            v_bf = v_bf_t[m] = kvp.tile([P, CIN], bf16, tag="v_bf",
                                        name="v_bf")
            nc.scalar.copy(v_bf[:], v_ps_t.pop(m)[:])

        # ---- dots + sumv for tile i-2 ----
        m2 = i - 2
        if 0 <= m2 < MT:
            khat = khat_t.pop(m2)
            v_bf = v_bf_t.pop(m2)
            for pr in range(NPAIR):
                mm = nc.tensor.matmul(
                    dots_ps[:, pr * P:(pr + 1) * P],
                    lhsT=khat[:, pr * P:(pr + 1) * P],
                    rhs=v_bf[:, pr * P:(pr + 1) * P],
                    start=(m2 == 0 and pr == 0),
                    stop=(m2 == MT - 1 and pr == NPAIR - 1))
                if m2 == 0:
                    if pr == 0:
                        dots_mm0 = mm
                    else:
                        add_dep_helper(mm.ins, dots_mm0.ins, sync=False,
                                       reason="psum group start order")
            nc.tensor.matmul(sumv_ps[:], lhsT=ones_bf[:], rhs=v_bf[:],
                             start=(m2 == 0), stop=(m2 == MT - 1))

    # flush remaining qhat^T transposes (chunks whose 4j+6 exceeds MT+1)
    for j in sorted(qh_ch):
        nc.sync.dma_start(out=qhatT[:, j], in_=qh_ch.pop(j)[:],
                          transpose=True)


def build_kernel():
    nc = bacc.Bacc(None, target_bir_lowering=False)
    x = nc.declare_dram_parameter("x", [NTOK, CIN], f32, isOutput=False)[:, :]
    w = nc.declare_dram_parameter("w_qkv", [CIN, N3], f32, isOutput=False)[:, :]
    gq = nc.declare_dram_parameter("q_gamma", [D], f32, isOutput=False)[:]
    bq = nc.declare_dram_parameter("q_beta", [D], f32, isOutput=False)[:]
    gk = nc.declare_dram_parameter("k_gamma", [D], f32, isOutput=False)[:]
    bk = nc.declare_dram_parameter("k_beta", [D], f32, isOutput=False)[:]
    out = nc.declare_dram_parameter("out", [NTOK, CIN], f32, isOutput=True)[:, :]

    with TileContext(nc) as tc:
        with tc.tile_pool(name="singles", bufs=1) as singles, \
             tc.tile_pool(name="xch", bufs=4) as xch, \
             tc.tile_pool(name="xTp", bufs=3) as xTp, \
             tc.tile_pool(name="qhp", bufs=3) as qhp, \
             tc.tile_pool(name="sqp", bufs=2) as sqp, \
             tc.tile_pool(name="stp", bufs=3) as stp, \
             tc.tile_pool(name="kvp", bufs=4) as kvp, \
             tc.tile_pool(name="outp", bufs=2) as outp:
            pools = (singles, xch, xTp, qhp, sqp, stp, kvp, outp)
            _body(nc, tc, pools, x, w, gq, bq, gk, bk, out)
    nc.compile()
    return nc


_LOCK = threading.Lock()
_CACHED = None


def _get_nc():
    global _CACHED
    with _LOCK:
        if _CACHED is None:
            _CACHED = build_kernel()
    return _CACHED


def kernel(x, w_qkv, q_gamma, q_beta, k_gamma, k_beta):
    from concourse.bass_utils import run_bass_kernel_spmd

    x = np.asarray(x, dtype=np.float32)
    w_qkv = np.asarray(w_qkv, dtype=np.float32)
    B, L, W, C = x.shape
    nc = _get_nc()
    in_maps = []
    for b in range(NCORES):
        in_maps.append({
            "x": np.ascontiguousarray(x[b].reshape(NTOK, CIN)),
            "w_qkv": w_qkv,
            "q_gamma": np.asarray(q_gamma, dtype=np.float32),
            "q_beta": np.asarray(q_beta, dtype=np.float32),
            "k_gamma": np.asarray(k_gamma, dtype=np.float32),
            "k_beta": np.asarray(k_beta, dtype=np.float32),
        })
    res = run_bass_kernel_spmd(nc, in_maps, list(range(NCORES)))
    out = np.stack([res.results[b]["out"] for b in range(NCORES)])
    return out.reshape(B, L, W, H * D).astype(np.float32)


# revision 9
# speedup vs baseline: 1.2259x; 1.2259x over previous
"""Trainium2 Bass kernel for nn_LinearAttentionBlock (linear attention).

Per-core (data-parallel over batch, 1 batch / core):
  x_b [4096, 512] -> qkv = x_b @ w_qkv -> per-head LayerNorm(q), LayerNorm(k)
  dots_h = LN(k)_h^T @ v_h   [64, 64]
  out_h  = LN(q)_h @ dots_h / 4096
  out    = concat_h(out_h)   [4096, 512]

v3 design (vs v2 at 202us, v1 at 218us):
  - Prologue on HWDGE: w and x chunks 0/1 loaded f32 (fast HW descriptors),
    converted to bf16 on-chip; per-chunk pipelined weight centering. First
    real matmul at ~10us instead of ~32us.
  - Engine rebalance: ACT evacuates q/k/v from PSUM to bf16 SBUF; GpSimd
    (otherwise idle) computes the squares and the rstd multiplies; DVE only
    does the segmented reduces + reciprocal.
  - Software pipeline: stats for tile i, rstd-muls for tile i-1, dots/sumv
    matmuls for tile i-2; x loads 4 chunks ahead, xT transposes 2 chunks
    ahead, qhat^T transposes 2 steps late -- every queue's FIFO head has
    pre-satisfied waits, so the PE never idles (HAM stays at full clock).
  - All xbar transposes serialized on the sync queue (concurrent transposes
    on two HWDGE queues corrupt packets -- found in v2).
  - Warm-up + keepalive dummy matmuls cover prologue and fixup phases.
  - beta_q correction: rank-1 K=1 matmul into the pass-3 PSUM group;
    evacuation alternates DVE/ACT plain copies.
"""
import threading

import numpy as np

import concourse.bacc as bacc
import concourse.bass as bass
import concourse.mybir as mybir
from concourse.tile import TileContext
from concourse.tile_rust import add_dep_helper

P = 128
NTOK = 4096          # tokens per batch (64*64)
CIN = 512            # input channels
N3 = 3 * CIN         # qkv columns
MT = NTOK // P       # 32 m-tiles
KC = CIN // P        # 4 k-chunks
H = 8                # heads
D = 64               # dim per head
NPAIR = H // 2       # 4 head pairs
CH = 4               # m-tiles per chunk
NCH = MT // CH       # 8 chunks
NCORES = 8
LN_EPS = 1e-5

f32 = mybir.dt.float32
bf16 = mybir.dt.bfloat16
X = mybir.AxisListType.X
MUL = mybir.AluOpType.mult
SUB = mybir.AluOpType.subtract
ADD = mybir.AluOpType.add


def _bc(ap, n):
    """Append a stride-0 broadcast dim of size n to an AP."""
    return bass.AP(ap.tensor, ap.offset, list(ap.ap) + [[0, n]])


def _col64(dram_ap):
    """View a [64] DRAM tensor as a [64, 1] column AP (partition-major)."""
    return bass.AP(dram_ap.tensor, dram_ap.offset, [[1, D], [1, 1]])


def _body(nc, tc, pools, x, w, gq, bq, gk, bk, out):
    singles, xch, xTp, qhp, sqp, stp, kvp, outp = pools

    # ---------------- P0: constants + weight prep ----------------
    ones_bf = singles.tile([P, P], bf16)
    nc.vector.memset(ones_bf[:], 1.0)

    # w: 4 chunked HWDGE f32 loads on the scalar queue, prep pipelined
    w_f32 = singles.tile([P, KC, N3], f32)
    for c in range(KC):
        nc.scalar.dma_start(
            out=w_f32[:, c, :],
            in_=w[c * P:(c + 1) * P, :])

    # x chunk staging (prologue chunks 0/1 via HWDGE f32 + ACT convert)
    x_tiles = {}
    xT_tiles = {}
    xst0 = singles.tile([P, CH, CIN], f32)
    xst1 = singles.tile([P, CH, CIN], f32)
    for j, xst in ((0, xst0), (1, xst1)):
        nc.sync.dma_start(
            out=xst[:],
            in_=x[j * CH * P:(j + 1) * CH * P, :].rearrange(
                "(t p) k -> p t k", p=P))
        t = xch.tile([P, CH, CIN], bf16, tag="x", name="x_ch")
        nc.scalar.copy(t[:], xst[:])
        x_tiles[j] = t

    def load_chunk(j):
        t = xch.tile([P, CH, CIN], bf16, tag="x", name="x_ch")
        nc.gpsimd.dma_start(
            out=t[:],
            in_=x[j * CH * P:(j + 1) * CH * P, :].rearrange(
                "(t p) k -> p t k", p=P))
        x_tiles[j] = t

    def transpose_chunk(j):
        t = xTp.tile([P, CH, KC, P], bf16, tag="xT", name="xT")
        nc.sync.dma_start(out=t[:], in_=x_tiles.pop(j)[:], transpose=True)
        xT_tiles[j] = t

    load_chunk(2)
    load_chunk(3)

    # per-chunk weight centering: wbar -> w_qk (bf16), w_v (bf16)
    wbar = singles.tile([P, KC, 2 * H], f32)
    w_qk = singles.tile([P, KC, 2 * CIN], bf16)
    w_v = singles.tile([P, KC, CIN], bf16)
    for c in range(KC):
        nc.vector.reduce_sum(
            wbar[:, c, :],
            w_f32[:, c, 0:2 * CIN].rearrange("p (g d) -> p g d", d=D),
            axis=X)
        nc.vector.tensor_scalar_mul(out=wbar[:, c, :], in0=wbar[:, c, :],
                                    scalar1=1.0 / D)
        nc.vector.tensor_tensor(
            out=w_qk[:, c, :].rearrange("p (g d) -> p g d", d=D),
            in0=w_f32[:, c, 0:2 * CIN].rearrange("p (g d) -> p g d", d=D),
            in1=_bc(wbar[:, c, :], D),
            op=SUB)
        nc.scalar.copy(w_v[:, c, :], w_f32[:, c, 2 * CIN:])

    transpose_chunk(0)
    transpose_chunk(1)

    # gamma/beta columns replicated onto both partition halves
    gq2 = singles.tile([P, 1], f32)
    gk2 = singles.tile([P, 1], f32)
    bk2 = singles.tile([P, 1], f32)
    for half in (0, 1):
        sl = slice(half * D, (half + 1) * D)
        nc.sync.dma_start(out=gq2[sl, :], in_=_col64(gq))
        nc.sync.dma_start(out=gk2[sl, :], in_=_col64(gk))
        nc.sync.dma_start(out=bk2[sl, :], in_=_col64(bk))
    bq_bf = singles.tile([D, 1], bf16)
    nc.gpsimd.dma_start(out=bq_bf[:], in_=_col64(bq))

    eps_t = singles.tile([P, 1], f32)
    nc.vector.memset(eps_t[:], float(D) * LN_EPS)
    d_all = singles.tile([P, NPAIR, P], bf16)
    nc.gpsimd.memset(d_all[:], 0.0)

    qhatT = singles.tile([P, NCH, CH, KC, P], bf16)

    with tc.tile_pool(name="ps_acc", bufs=1, space="PSUM") as ps_acc:
        dots_ps = ps_acc.tile([P, 4 * P], f32)
        sumv_ps = ps_acc.tile([P, CIN], f32)
        with tc.tile_pool(name="ps_qkv", bufs=2, space="PSUM") as ps_qkv:
            # HAM warm-up burst (no data deps -> runs immediately), then
            # keepalives pinned to late-prologue producers. Targets the
            # sumv bank; the real sumv group re-clears it with start=True.
            for _ in range(44):
                nc.tensor.matmul(sumv_ps[:, 0:P], lhsT=ones_bf[:],
                                 rhs=ones_bf[:], start=True, stop=True)
            for c in range(KC):
                for _ in range(2):
                    nc.tensor.matmul(sumv_ps[:, 0:P], lhsT=ones_bf[:],
                                     rhs=w_qk[:, c, 0:P],
                                     start=True, stop=True)
            for _ in range(2):
                nc.tensor.matmul(sumv_ps[:, 0:P], lhsT=ones_bf[:],
                                 rhs=xT_tiles[0][:, 0, 0, :],
                                 start=True, stop=True)

            _p1_loop(nc, load_chunk, transpose_chunk, xT_tiles,
                     w_qk, w_v, eps_t, ones_bf, qhatT,
                     dots_ps, sumv_ps, ps_qkv, qhp, sqp, stp, kvp)

        # ---------------- P2: dots fixups ----------------
        dots_sb = singles.tile([P, 4 * P], f32)
        nc.vector.tensor_copy(out=dots_sb[:], in_=dots_ps[:])
        sumv_sb = singles.tile([P, CIN], f32)
        nc.vector.tensor_copy(out=sumv_sb[:], in_=sumv_ps[:])

    ktmp = singles.tile([P, NPAIR, D], f32)
    bsum = singles.tile([P, NPAIR, D], f32)
    deo = singles.tile([P, NPAIR, D], f32)
    for half in (0, 1):
        sl = slice(half * D, (half + 1) * D)
        # KV diag block, scaled by gamma_k * 8
        nc.vector.tensor_scalar(
            out=ktmp[sl, :, :],
            in0=dots_sb[sl, :].rearrange("p (pr x) -> p pr x", x=P)[
                :, :, half * D:(half + 1) * D],
            scalar1=gk2[sl, :], scalar2=8.0, op0=MUL, op1=MUL)
        # beta_k (x) sumV
        nc.vector.tensor_scalar(
            out=bsum[sl, :, :],
            in0=sumv_sb[sl, :].rearrange(
                "p (pr two d) -> p pr two d", two=2, d=D)[:, :, half, :],
            scalar1=bk2[sl, :], scalar2=None, op0=MUL)
    nc.vector.tensor_add(deo[:], ktmp[:], bsum[:])

    for half in (0, 1):
        sl = slice(half * D, (half + 1) * D)
        nc.vector.tensor_scalar(
            out=d_all[sl, :, half * D:(half + 1) * D],
            in0=deo[sl, :, :],
            scalar1=gq2[sl, :], scalar2=8.0 / NTOK, op0=MUL, op1=MUL)

    # c row: beta_q @ dots / NTOK (rank-1 correction, added in pass 3)
    dstack = singles.tile([D, H, D], bf16)
    nc.vector.tensor_copy(
        out=dstack.rearrange("p (pr two) d -> p pr two d", two=2)[:, :, 0, :],
        in_=deo[0:D, :, :])
    nc.gpsimd.dma_start(
        out=dstack.rearrange("p (pr two) d -> p pr two d", two=2)[:, :, 1, :],
        in_=deo[D:P, :, :])

    with tc.tile_pool(name="ps_fix", bufs=1, space="PSUM") as ps_fix, \
         tc.tile_pool(name="ps_out", bufs=2, space="PSUM") as ps_out, \
         tc.tile_pool(name="ps_warm2", bufs=1, space="PSUM") as ps_warm2:
        # keepalive through the fixup phase (no >3.4us PE idle)
        warm2 = ps_warm2.tile([P, P], f32)
        nc.tensor.matmul(warm2[:], lhsT=ones_bf[:], rhs=d_all[:, 0, :],
                         start=True, stop=True)

        c_ps = ps_fix.tile([1, CIN], f32)
        nc.tensor.matmul(c_ps[:], lhsT=bq_bf[:],
                         rhs=dstack.rearrange("p h d -> p (h d)"),
                         start=True, stop=True)
        c_bf = singles.tile([1, CIN], bf16)
        nc.vector.tensor_scalar_mul(out=c_bf[:], in0=c_ps[:],
                                    scalar1=1.0 / NTOK)
        nc.tensor.matmul(warm2[0:1, :], lhsT=ones_bf[0:1, 0:1],
                         rhs=c_bf[:, 0:P], start=True, stop=True)

        # ------------ P3: out = qhat @ D (pair blockdiag) + 1 (x) c ------
        for ci in range(NCH):
            out_ch = outp.tile([P, CH, CIN], f32, tag="out", name="out_ch")
            for tt in range(CH):
                o_ps = ps_out.tile([P, CIN], f32, tag="o", name="o_ps")
                mm0 = None
                for pr in range(NPAIR):
                    mm = nc.tensor.matmul(
                        o_ps[:, pr * P:(pr + 1) * P],
                        lhsT=qhatT[:, ci, tt, pr, :],
                        rhs=d_all[:, pr, :],
                        start=(pr == 0), stop=False)
                    if pr == 0:
                        mm0 = mm
                    else:
                        add_dep_helper(mm.ins, mm0.ins, sync=False,
                                       reason="psum group start order")
                mmc = nc.tensor.matmul(
                    o_ps[:], lhsT=ones_bf[0:1, :], rhs=c_bf[:],
                    start=False, stop=True)
                add_dep_helper(mmc.ins, mm0.ins, sync=False,
                               reason="psum group start order")
                if tt % 2 == 0:
                    nc.vector.tensor_copy(out=out_ch[:, tt, :], in_=o_ps[:])
                else:
                    nc.scalar.copy(out_ch[:, tt, :], o_ps[:])
            nc.sync.dma_start(
                out=out[ci * CH * P:(ci + 1) * CH * P, :].rearrange(
                    "(t p) k -> p t k", p=P),
                in_=out_ch[:])


def _p1_loop(nc, load_chunk, transpose_chunk, xT_tiles,
             w_qk, w_v, eps_t, ones_bf, qhatT,
             dots_ps, sumv_ps, ps_qkv, qhp, sqp, stp, kvp):
    q_ps_t = {}
    k_ps_t = {}
    v_ps_t = {}
    rstd_t = {}
    khat_t = {}
    v_bf_t = {}
    qh_ch = {}
    dots_mm0 = None

    for i in range(MT + 2):
        # chunk machinery: loads 4 chunks ahead, xT 2 chunks ahead,
        # qhat^T transposes 2 steps after their chunk completes.
        if i % CH == 0 and i < MT:
            jj = i // CH
            if jj + 4 < NCH:
                load_chunk(jj + 4)
            if jj + 2 < NCH:
                transpose_chunk(jj + 2)
        if i >= 6 and (i - 6) % CH == 0:
            jdone = (i - 6) // CH
            if jdone in qh_ch:
                nc.sync.dma_start(out=qhatT[:, jdone],
                                  in_=qh_ch.pop(jdone)[:], transpose=True)

        if i < MT:
            j, tt = divmod(i, CH)
            if tt == 0:
                qh_ch[j] = qhp.tile([P, CH, CIN], bf16, tag="qh",
                                    name="qh_ch")

            # ---- QKV matmuls for tile i ----
            xT = xT_tiles[j]
            psq = ps_qkv.tile([P, CIN], f32, tag="q", name="q_ps")
            psk = ps_qkv.tile([P, CIN], f32, tag="k", name="k_ps")
            psv = v_ps_t[i] = ps_qkv.tile([P, CIN], f32, tag="v", name="v_ps")
            for pst, rhs_tile, base in ((psq, w_qk, 0),
                                        (psk, w_qk, CIN),
                                        (psv, w_v, 0)):
                for c in range(KC):
                    nc.tensor.matmul(
                        pst[:], lhsT=xT[:, tt, c, :],
                        rhs=rhs_tile[:, c, base:base + CIN],
                        start=(c == 0), stop=(c == KC - 1))

            # ---- squares (ACT, PSUM->bf16 SBUF), stats (DVE) ----
            q_ps_t[i] = psq
            k_ps_t[i] = psk
            sq2 = sqp.tile([P, 2, CIN], bf16, tag="sq2", name="sq2")
            nc.scalar.square(sq2[:, 0, :], psq[:])
            nc.scalar.square(sq2[:, 1, :], psk[:])
            st = stp.tile([P, 2, H], f32, tag="st", name="st")
            nc.vector.reduce_sum(
                st[:], sq2.rearrange("p two (h d) -> p two h d", d=D), axis=X)
            rstd = rstd_t[i] = stp.tile([P, 2, H], f32, tag="rstd",
                                        name="rstd")
            nc.scalar.activation(
                out=rstd[:], in_=st[:],
                func=mybir.ActivationFunctionType.Sqrt,
                bias=eps_t[:], scale=1.0)
            nc.vector.reciprocal(rstd[:], rstd[:])

        # ---- rstd application for tile i-1 (DVE) + v evac (ACT) ----
        m = i - 1
        if 0 <= m < MT:
            jm, ttm = divmod(m, CH)
            rstd = rstd_t.pop(m)
            nc.vector.tensor_tensor(
                out=qh_ch[jm][:, ttm, :].rearrange("p (h d) -> p h d", d=D),
                in0=q_ps_t.pop(m).rearrange("p (h d) -> p h d", d=D),
                in1=_bc(rstd[:, 0, :], D), op=MUL)
            khat = khat_t[m] = kvp.tile([P, CIN], bf16, tag="khat",
                                        name="khat")
            nc.vector.tensor_tensor(
                out=khat.rearrange("p (h d) -> p h d", d=D),
                in0=k_ps_t.pop(m).rearrange("p (h d) -> p h d", d=D),
                in1=_bc(rstd[:, 1, :], D), op=MUL)
            v_bf = v_bf_t[m] = kvp.tile([P, CIN], bf16, tag="v_bf",
                                        name="v_bf")
            nc.scalar.copy(v_bf[:], v_ps_t.pop(m)[:])

        # ---- dots + sumv for tile i-2 ----
        m2 = i - 2
        if 0 <= m2 < MT:
            khat = khat_t.pop(m2)
            v_bf = v_bf_t.pop(m2)
            for pr in range(NPAIR):
                mm = nc.tensor.matmul(
                    dots_ps[:, pr * P:(pr + 1) * P],
                    lhsT=khat[:, pr * P:(pr + 1) * P],
                    rhs=v_bf[:, pr * P:(pr + 1) * P],
                    start=(m2 == 0 and pr == 0),
                    stop=(m2 == MT - 1 and pr == NPAIR - 1))
                if m2 == 0:
                    if pr == 0:
                        dots_mm0 = mm
                    else:
                        add_dep_helper(mm.ins, dots_mm0.ins, sync=False,
                                       reason="psum group start order")
            nc.tensor.matmul(sumv_ps[:], lhsT=ones_bf[:], rhs=v_bf[:],
                             start=(m2 == 0), stop=(m2 == MT - 1))

    # flush remaining qhat^T transposes (chunks whose 4j+6 exceeds MT+1)
    for j in sorted(qh_ch):
        nc.sync.dma_start(out=qhatT[:, j], in_=qh_ch.pop(j)[:],
                          transpose=True)


def build_kernel():
    nc = bacc.Bacc(None, target_bir_lowering=False)
    x = nc.declare_dram_parameter("x", [NTOK, CIN], f32, isOutput=False)[:, :]
    w = nc.declare_dram_parameter("w_qkv", [CIN, N3], f32, isOutput=False)[:, :]
    gq = nc.declare_dram_parameter("q_gamma", [D], f32, isOutput=False)[:]
    bq = nc.declare_dram_parameter("q_beta", [D], f32, isOutput=False)[:]
    gk = nc.declare_dram_parameter("k_gamma", [D], f32, isOutput=False)[:]
    bk = nc.declare_dram_parameter("k_beta", [D], f32, isOutput=False)[:]
    out = nc.declare_dram_parameter("out", [NTOK, CIN], f32, isOutput=True)[:, :]

    with TileContext(nc) as tc:
        with tc.tile_pool(name="singles", bufs=1) as singles, \
             tc.tile_pool(name="xch", bufs=4) as xch, \
             tc.tile_pool(name="xTp", bufs=3) as xTp, \
             tc.tile_pool(name="qhp", bufs=3) as qhp, \
             tc.tile_pool(name="sqp", bufs=2) as sqp, \
             tc.tile_pool(name="stp", bufs=3) as stp, \
             tc.tile_pool(name="kvp", bufs=4) as kvp, \
             tc.tile_pool(name="outp", bufs=2) as outp:
            pools = (singles, xch, xTp, qhp, sqp, stp, kvp, outp)
            _body(nc, tc, pools, x, w, gq, bq, gk, bk, out)
    nc.compile()
    return nc


_LOCK = threading.Lock()
_CACHED = None


def _get_nc():
    global _CACHED
    with _LOCK:
        if _CACHED is None:
            _CACHED = build_kernel()
    return _CACHED


def kernel(x, w_qkv, q_gamma, q_beta, k_gamma, k_beta):
    from concourse.bass_utils import run_bass_kernel_spmd

    x = np.asarray(x, dtype=np.float32)
    w_qkv = np.asarray(w_qkv, dtype=np.float32)
    B, L, W, C = x.shape
    nc = _get_nc()
    in_maps = []
    for b in range(NCORES):
        in_maps.append({
            "x": np.ascontiguousarray(x[b].reshape(NTOK, CIN)),
            "w_qkv": w_qkv,
            "q_gamma": np.asarray(q_gamma, dtype=np.float32),
            "q_beta": np.asarray(q_beta, dtype=np.float32),
            "k_gamma": np.asarray(k_gamma, dtype=np.float32),
            "k_beta": np.asarray(k_beta, dtype=np.float32),
        })
    res = run_bass_kernel_spmd(nc, in_maps, list(range(NCORES)))
    out = np.stack([res.results[b]["out"] for b in range(NCORES)])
    return out.reshape(B, L, W, H * D).astype(np.float32)
